# Initial kernel scaffold
#
"""Trainium2 Bass kernel for nn_BoundaryPredictor3 (segment_reduce).

Strategy (see test.py for the validation harness):
  - 8 cores: core c handles batch c//2 and output-segment half c%2.
  - Per core: boundary-MLP pipeline in f32 on-chip, exact replication of the
    executed-reference's float32 "dirt" semantics for boundaries
    (boundaries = (hard + p) - p, cumsum accumulated exactly then rounded
    once, f32-exact onehot compare), then segment softmax pooling via
    onehot matmuls on the PE.
  - Host only folds parameters (transposes, Wq.T@Wk, broadcast of the
    learned query) and computes the 4 length scalars.
"""
import sys
import os
sys.path.insert(0, '/opt/trn_rl_repo')

import numpy as np
from contextlib import ExitStack

B, L, D, H, HD = 4, 1024, 512, 8, 64
NT = L // 128            # 8 l-tiles
NC = D // 128            # 4 d-chunks
N_CORES = 8
SPLIT = 2                # s-halves per batch
SH = L // SPLIT          # 512 segment slots per core
ST = SH // 128           # 4 s-tiles per core
EPS_L2 = 1e-8
LN_EPS = 1e-5

_BUILD_CACHE = {}


def _build(use_mqk, use_b1, use_b2):
    import concourse.bass as bass
    import concourse.bacc as bacc
    import concourse.tile as tile
    from concourse import mybir
    from concourse.masks import make_identity

    F32 = mybir.dt.float32
    OP = mybir.AluOpType
    AF = mybir.ActivationFunctionType

    nc = bacc.Bacc("TRN2", target_bir_lowering=False, debug=False,
                   enable_asserts=True, num_devices=N_CORES)

    hid = nc.dram_tensor("hidden", [L, D], F32, kind="ExternalInput").ap()
    w1t = nc.dram_tensor("w1t", [D, D], F32, kind="ExternalInput").ap()
    w2t = nc.dram_tensor("w2t", [D, D], F32, kind="ExternalInput").ap()
    qbc_d = nc.dram_tensor("qbc", [128, D], F32, kind="ExternalInput").ap()
    lm_d = nc.dram_tensor("lenmask", [L], F32, kind="ExternalInput").ap()
    iota_d = nc.dram_tensor("iota_b", [1, SH], F32, kind="ExternalInput").ap()
    triu_d = nc.dram_tensor("triu", [128, 128], F32, kind="ExternalInput").ap()
    shf_d = nc.dram_tensor("shift1", [128, 128], F32, kind="ExternalInput").ap()
    e127_d = nc.dram_tensor("e127", [128, 128], F32, kind="ExternalInput").ap()
    if use_mqk:
        mqk_d = nc.dram_tensor("mqk", [D, D], F32, kind="ExternalInput").ap()
    if use_b1:
        b1_d = nc.dram_tensor("b1row", [1, D], F32, kind="ExternalInput").ap()
    if use_b2:
        b2_d = nc.dram_tensor("b2row", [1, D], F32, kind="ExternalInput").ap()
    out_d = nc.dram_tensor("out", [SH, D], F32, kind="ExternalOutput").ap()

    with ExitStack() as ctx:
        tc = ctx.enter_context(tile.TileContext(nc))
        cst = ctx.enter_context(tc.tile_pool(name="cst", bufs=1))
        big = ctx.enter_context(tc.tile_pool(name="big", bufs=1))
        wrk = ctx.enter_context(tc.tile_pool(name="wrk", bufs=3))
        sml = ctx.enter_context(tc.tile_pool(name="sml", bufs=2))
        psb = ctx.enter_context(tc.tile_pool(name="psb", bufs=3, space="PSUM"))
        pss = ctx.enter_context(tc.tile_pool(name="pss", bufs=2, space="PSUM"))

        # ---------------- constants ----------------
        ident = cst.tile([128, 128], F32, tag="ident")
        make_identity(nc, ident)
        w1s = cst.tile([128, NC, NC, 128], F32, tag="w1s")   # [k, kc, jt, m]
        nc.sync.dma_start(out=w1s, in_=w1t.rearrange(
            "(kc k) (jt m) -> k kc jt m", k=128, m=128))
        w2s = cst.tile([128, NC, D], F32, tag="w2s")         # [k, kc, n]
        nc.sync.dma_start(out=w2s, in_=w2t.rearrange("(kc k) n -> k kc n", k=128))
        if use_mqk:
            mqs = cst.tile([128, NC, NC, 128], F32, tag="mqs")
            nc.sync.dma_start(out=mqs, in_=mqk_d.rearrange(
                "(kc k) (jt m) -> k kc jt m", k=128, m=128))
        qbc = cst.tile([128, D], F32, tag="qbc")
        nc.sync.dma_start(out=qbc, in_=qbc_d)
        lm8 = cst.tile([128, NT], F32, tag="lm8")            # col-major lenmask
        nc.sync.dma_start(out=lm8, in_=lm_d.rearrange("(t p) -> p t", p=128))
        triu = cst.tile([128, 128], F32, tag="triu")
        nc.sync.dma_start(out=triu, in_=triu_d)
        shf = cst.tile([128, 128], F32, tag="shf")
        nc.sync.dma_start(out=shf, in_=shf_d)
        e127 = cst.tile([128, 128], F32, tag="e127")
        nc.sync.dma_start(out=e127, in_=e127_d)
        iota_r = cst.tile([1, SH], F32, tag="iota_r")
        nc.sync.dma_start(out=iota_r, in_=iota_d)
        ones_r128 = cst.tile([1, 128], F32, tag="ones_r128")
        nc.vector.memset(ones_r128, 1.0)
        ones_c = cst.tile([128, 1], F32, tag="ones_c")
        nc.vector.memset(ones_c, 1.0)
        if use_b1 or use_b2:
            ones_r512 = cst.tile([1, D], F32, tag="ones_r512")
            nc.vector.memset(ones_r512, 1.0)
        if use_b1:
            b1r = cst.tile([1, D], F32, tag="b1r")
            nc.sync.dma_start(out=b1r, in_=b1_d)
        if use_b2:
            b2r = cst.tile([1, D], F32, tag="b2r")
            nc.sync.dma_start(out=b2r, in_=b2_d)

        # iota broadcast [128, SH] via K=1 matmul
        ps_io = psb.tile([128, SH], F32, tag="ps_big")
        nc.tensor.matmul(ps_io, ones_r128, iota_r, start=True, stop=True)
        iota_b = cst.tile([128, SH], F32, tag="iota_b")
        nc.vector.tensor_copy(iota_b, ps_io)

        def newton_invsqrt(s_cols, seed, iters, name):
            """y ~= 1/sqrt(s_cols), Newton from constant seed (DVE only)."""
            y = sml.tile([128, NT], F32, tag=f"nw_y_{name}")
            nc.vector.memset(y, seed)
            for i in range(iters):
                t = sml.tile([128, NT], F32, tag=f"nw_t_{name}")
                nc.vector.tensor_tensor(out=t, in0=y, in1=y, op=OP.mult)
                nc.vector.tensor_tensor(out=t, in0=t, in1=s_cols, op=OP.mult)
                nc.vector.tensor_scalar(out=t, in0=t, scalar1=-0.5, scalar2=1.5,
                                        op0=OP.mult, op1=OP.add)
                y2 = sml.tile([128, NT], F32, tag=f"nw_y_{name}")
                nc.vector.tensor_tensor(out=y2, in0=y, in1=t, op=OP.mult)
                y = y2
            return y

        # ---------------- stage 1: load + norms ----------------
        X = big.tile([128, NT, D], F32, tag="X")
        nc.sync.dma_start(out=X, in_=hid.rearrange("(t p) d -> p t d", p=128))

        ss = sml.tile([128, NT], F32, tag="ss")
        sq_scr = wrk.tile([128, D], F32, tag="sq_scr")
        for t in range(NT):
            scr = wrk.tile([128, D], F32, tag="sq_scr")
            nc.scalar.activation(scr, X[:, t, :], AF.Square,
                                 accum_out=ss[:, t:t + 1])
        inv_c = newton_invsqrt(ss, 1.0 / np.sqrt(512.0), 4, "x")

        N8 = big.tile([128, NT, D], F32, tag="N8")
        for t in range(NT):
            nc.vector.tensor_scalar(out=N8[:, t, :], in0=X[:, t, :],
                                    scalar1=inv_c[:, t:t + 1], scalar2=None,
                                    op0=OP.mult)

        # layernorm stats
        mv = sml.tile([128, NT, 2], F32, tag="mv")
        for t in range(NT):
            stt = wrk.tile([128, 6], F32, tag="bn_st")
            nc.vector.bn_stats(out=stt, in_=X[:, t, :])
            nc.vector.bn_aggr(out=mv[:, t, :], in_=stt)
        veps = sml.tile([128, NT], F32, tag="veps")
        nc.vector.tensor_scalar(out=veps, in0=mv[:, :, 1], scalar1=LN_EPS,
                                scalar2=None, op0=OP.add)
        rstd = newton_invsqrt(veps, 1.0, 4, "v")

        HN = big.tile([128, NT, D], F32, tag="HN")
        for t in range(NT):
            nc.vector.tensor_scalar(out=HN[:, t, :], in0=X[:, t, :],
                                    scalar1=mv[:, t, 0:1],
                                    scalar2=rstd[:, t:t + 1],
                                    op0=OP.subtract, op1=OP.mult)

        # ---------------- stage 2: NT transposes ----------------
        NTt = big.tile([128, NC, L], F32, tag="NTt")
        for dc in range(NC):
            for half in range(2):
                pst = psb.tile([128, 512], F32, tag="ps_big")
                for i in range(4):
                    t = half * 4 + i
                    nc.tensor.transpose(pst[:, i * 128:(i + 1) * 128],
                                        N8[:, t, dc * 128:(dc + 1) * 128], ident)
                nc.scalar.copy(NTt[:, dc, half * 512:(half + 1) * 512], pst)

        # ---------------- stage 3: y1T = W1 @ n^T, gelu ----------------
        y1gT = big.tile([128, NC, L], F32, tag="y1gT")
        for jt in range(NC):
            for nch in range(2):
                ps = psb.tile([128, 512], F32, tag="ps_big")
                for kc in range(NC):
                    nc.tensor.matmul(ps, w1s[:, kc, jt, :],
                                     NTt[:, kc, nch * 512:(nch + 1) * 512],
                                     start=(kc == 0), stop=(kc == NC - 1 and not use_b1))
                if use_b1:
                    nc.tensor.matmul(ps, b1r[:, jt * 128:(jt + 1) * 128],
                                     ones_r512[:, 0:512], start=False, stop=True)
                nc.scalar.activation(y1gT[:, jt, nch * 512:(nch + 1) * 512],
                                     ps, AF.Gelu)

        # ---------------- stage 4: y2 + residual ----------------
        R8 = big.tile([128, NT, D], F32, tag="R8")
        for lt in range(NT):
            ps = psb.tile([128, 512], F32, tag="ps_big")
            for kc in range(NC):
                nc.tensor.matmul(ps, y1gT[:, kc, lt * 128:(lt + 1) * 128],
                                 w2s[:, kc, :],
                                 start=(kc == 0), stop=(kc == NC - 1 and not use_b2))
            if use_b2:
                nc.tensor.matmul(ps, ones_r128, b2r, start=False, stop=True)
            nc.vector.tensor_tensor(out=R8[:, lt, :], in0=ps, in1=N8[:, lt, :],
                                    op=OP.add)

        # ---------------- stage 5: l2norm scalars of R ----------------
        ssr = sml.tile([128, NT], F32, tag="ssr")
        for t in range(NT):
            scr = wrk.tile([128, D], F32, tag="sq_scr")
            nc.scalar.activation(scr, R8[:, t, :], AF.Square,
                                 accum_out=ssr[:, t:t + 1])
        invr = newton_invsqrt(ssr, 1.0, 4, "r")

        # ---------------- stage 6: RT transposes ----------------
        RT = big.tile([128, NC, L], F32, tag="RT")
        for dc in range(NC):
            for half in range(2):
                pst = psb.tile([128, 512], F32, tag="ps_big")
                for i in range(4):
                    t = half * 4 + i
                    nc.tensor.transpose(pst[:, i * 128:(i + 1) * 128],
                                        R8[:, t, dc * 128:(dc + 1) * 128], ident)
                nc.scalar.copy(RT[:, dc, half * 512:(half + 1) * 512], pst)

        # ---------------- stage 7: optional Mqk fold ----------------
        if use_mqk:
            ART = big.tile([128, NC, L], F32, tag="ART")
            for jt in range(NC):
                for nch in range(2):
                    ps = psb.tile([128, 512], F32, tag="ps_big")
                    for kc in range(NC):
                        nc.tensor.matmul(ps, mqs[:, kc, jt, :],
                                         RT[:, kc, nch * 512:(nch + 1) * 512],
                                         start=(kc == 0), stop=(kc == NC - 1))
                    nc.scalar.copy(ART[:, jt, nch * 512:(nch + 1) * 512], ps)
        else:
            ART = RT

        # ---------------- stage 8: cos ----------------
        prod = big.tile([128, NC, L], F32, tag="prod")
        for dc in range(NC):
            nc.vector.tensor_tensor(out=prod[:, dc, 0:L - 1],
                                    in0=ART[:, dc, 0:L - 1],
                                    in1=RT[:, dc, 1:L], op=OP.mult)
            nc.vector.memset(prod[:, dc, L - 1:L], 0.0)
        ps_raw = pss.tile([128, NT], F32, tag="ps_sml")
        for t in range(NT):
            for dc in range(NC):
                nc.tensor.matmul(ps_raw[:, t:t + 1],
                                 prod[:, dc, t * 128:(t + 1) * 128], ones_c,
                                 start=(dc == 0), stop=(dc == NC - 1))
        ps_shift = pss.tile([128, NT], F32, tag="ps_sml")
        nc.tensor.matmul(ps_shift, shf, invr, start=True, stop=False)
        nc.tensor.matmul(ps_shift[:, 0:NT - 1], e127, invr[:, 1:NT],
                         start=False, stop=True)
        cosr = sml.tile([128, NT], F32, tag="cosr")
        nc.vector.tensor_tensor(out=cosr, in0=ps_raw, in1=invr, op=OP.mult)
        cos_c = sml.tile([128, NT], F32, tag="cos_c")
        nc.vector.tensor_tensor(out=cos_c, in0=cosr, in1=ps_shift, op=OP.mult)

        # ---------------- stage 9: boundary chain (exact f32 dirt) --------
        probs = sml.tile([128, NT], F32, tag="probs")
        # (cos - 1) * -0.5 == fl((1-cos)*0.5) bitwise
        nc.vector.tensor_scalar(out=probs, in0=cos_c, scalar1=1.0, scalar2=-0.5,
                                op0=OP.subtract, op1=OP.mult)
        nc.vector.tensor_scalar(out=probs, in0=probs, scalar1=0.0, scalar2=1.0,
                                op0=OP.max, op1=OP.min)
        nc.vector.memset(probs[127:128, NT - 1:NT], 0.0)     # pad l = L-1
        hard = sml.tile([128, NT], F32, tag="hard")
        nc.vector.tensor_scalar(out=hard, in0=probs, scalar1=0.5, scalar2=None,
                                op0=OP.is_gt)
        thp = sml.tile([128, NT], F32, tag="thp")
        nc.vector.tensor_tensor(out=thp, in0=hard, in1=probs, op=OP.add)
        bexec = sml.tile([128, NT], F32, tag="bexec")
        nc.vector.tensor_tensor(out=bexec, in0=thp, in1=probs, op=OP.subtract)
        dirt = sml.tile([128, NT], F32, tag="dirt")
        nc.vector.tensor_tensor(out=dirt, in0=hard, in1=bexec, op=OP.subtract)

        def carry_excl(cols, name):
            """[1, NT] exclusive cumsum of column totals of cols [128, NT]."""
            ps_tot = pss.tile([1, NT], F32, tag="ps_tot")
            nc.tensor.matmul(ps_tot, ones_c, cols, start=True, stop=True)
            a = sml.tile([1, NT], F32, tag=f"ce_a_{name}")
            nc.vector.memset(a[:, 0:1], 0.0)
            nc.vector.tensor_copy(a[:, 1:NT], ps_tot[:, 0:NT - 1])
            for sh_ in (1, 2, 4):
                b = sml.tile([1, NT], F32, tag=f"ce_a_{name}")
                nc.vector.tensor_copy(b[:, 0:sh_], a[:, 0:sh_])
                nc.vector.tensor_tensor(out=b[:, sh_:NT], in0=a[:, sh_:NT],
                                        in1=a[:, 0:NT - sh_], op=OP.add)
                a = b
            return a

        carH = carry_excl(hard, "h")
        carD = carry_excl(dirt, "d")
        psK = pss.tile([128, NT], F32, tag="ps_sml")
        nc.tensor.matmul(psK, triu, hard, start=True, stop=False)
        nc.tensor.matmul(psK, ones_r128, carH, start=False, stop=True)
        psD = pss.tile([128, NT], F32, tag="ps_sml")
        nc.tensor.matmul(psD, triu, dirt, start=True, stop=False)
        nc.tensor.matmul(psD, ones_r128, carD, start=False, stop=True)
        sbK = sml.tile([128, NT], F32, tag="sbK")
        nc.vector.tensor_copy(sbK, psK)
        cum = sml.tile([128, NT], F32, tag="cum")
        nc.vector.tensor_tensor(out=cum, in0=sbK, in1=psD, op=OP.subtract)
        seg = sml.tile([128, NT], F32, tag="seg")
        nc.vector.tensor_tensor(out=seg, in0=cum, in1=bexec, op=OP.subtract)
        # mask positions beyond actual length: push seg out of range
        lmB = sml.tile([128, NT], F32, tag="lmB")
        nc.vector.tensor_scalar(out=lmB, in0=lm8, scalar1=-1e9, scalar2=1e9,
                                op0=OP.mult, op1=OP.add)
        segm = sml.tile([128, NT], F32, tag="segm")
        nc.vector.tensor_tensor(out=segm, in0=seg, in1=lmB, op=OP.add)

        # ---------------- stage 10: scores + E ----------------
        sc8 = sml.tile([128, NT, H], F32, tag="sc8")
        for t in range(NT):
            tmp = wrk.tile([128, D], F32, tag="sc_tmp")
            nc.vector.tensor_tensor(out=tmp, in0=HN[:, t, :], in1=qbc, op=OP.mult)
            nc.vector.tensor_reduce(out=sc8[:, t, :],
                                    in_=tmp.rearrange("p (h j) -> p h j", h=H),
                                    axis=mybir.AxisListType.X, op=OP.add)
        E8 = sml.tile([128, NT, H], F32, tag="E8")
        nc.scalar.activation(E8, sc8, AF.Exp)

        # ---------------- stage 11: Xp = E (bcast) * HN ----------------
        import concourse.bass as bass_mod
        Xp = big.tile([128, NT, D], F32, tag="Xp")
        for t in range(NT):
            e_t = E8[:, t, :]
            ebc = bass_mod.AP(tensor=e_t.tensor, offset=e_t.offset,
                              ap=[e_t.ap[0], [e_t.ap[1][0], H], [0, HD]])
            nc.vector.tensor_tensor(
                out=Xp[:, t, :].rearrange("p (h j) -> p h j", h=H),
                in0=HN[:, t, :].rearrange("p (h j) -> p h j", h=H),
                in1=ebc, op=OP.mult)

        # ---------------- stage 12: onehot + pooling ----------------
        oh = big.tile([128, NT, SH], F32, tag="oh")
        for t in range(NT):
            nc.vector.tensor_scalar(out=oh[:, t, :], in0=iota_b,
                                    scalar1=segm[:, t:t + 1], scalar2=None,
                                    op0=OP.is_equal)
        for st in range(ST):
            psP = psb.tile([128, 512], F32, tag="ps_big")
            psZ = pss.tile([128, H], F32, tag="ps_sml")
            for t in range(NT):
                nc.tensor.matmul(psP, oh[:, t, st * 128:(st + 1) * 128],
                                 Xp[:, t, :], start=(t == 0), stop=(t == NT - 1))
            for t in range(NT):
                nc.tensor.matmul(psZ, oh[:, t, st * 128:(st + 1) * 128],
                                 E8[:, t, :], start=(t == 0), stop=(t == NT - 1))
            ispos = sml.tile([128, H], F32, tag="ispos")
            nc.vector.tensor_scalar(out=ispos, in0=psZ, scalar1=0.0,
                                    scalar2=None, op0=OP.is_gt)
            notp = sml.tile([128, H], F32, tag="notp")
            nc.vector.tensor_scalar(out=notp, in0=ispos, scalar1=-1.0,
                                    scalar2=1.0, op0=OP.mult, op1=OP.add)
            zsafe = sml.tile([128, H], F32, tag="zsafe")
            nc.vector.tensor_tensor(out=zsafe, in0=psZ, in1=notp, op=OP.add)
            rz = sml.tile([128, H], F32, tag="rz")
            nc.vector.reciprocal(rz, zsafe)
            nc.vector.tensor_tensor(out=rz, in0=rz, in1=ispos, op=OP.mult)
            pooled = wrk.tile([128, D], F32, tag="pooled")
            rzb = bass_mod.AP(tensor=rz.tensor, offset=rz.offset,
                              ap=[rz.ap[0], [rz.ap[1][0], H], [0, HD]])
            nc.vector.tensor_tensor(
                out=pooled.rearrange("p (h j) -> p h j", h=H),
                in0=psP.rearrange("p (h j) -> p h j", h=H),
                in1=rzb, op=OP.mult)
            nc.sync.dma_start(out=out_d[st * 128:(st + 1) * 128, :], in_=pooled)

    nc.finalize()
    return nc


def kernel(**inputs):
    from concourse.bass_utils import run_bass_kernel_spmd

    hidden = np.ascontiguousarray(np.asarray(inputs["hidden"], dtype=np.float32))
    lengths = np.asarray(inputs["lengths"], dtype=np.float32)
    W1 = np.asarray(inputs["W1"], dtype=np.float32)
    b1 = np.asarray(inputs["b1"], dtype=np.float32)
    W2 = np.asarray(inputs["W2"], dtype=np.float32)
    b2 = np.asarray(inputs["b2"], dtype=np.float32)
    Wq = np.asarray(inputs["Wq"], dtype=np.float32)
    Wk = np.asarray(inputs["Wk"], dtype=np.float32)
    sim_bias = float(np.asarray(inputs["sim_bias"], dtype=np.float32))
    lq = np.asarray(inputs["learned_query"], dtype=np.float32)
    Wpk = np.asarray(inputs["Wpk"], dtype=np.float32)
    Wpv = np.asarray(inputs["Wpv"], dtype=np.float32)
    Wpo = np.asarray(inputs["Wpo"], dtype=np.float32)
    gamma = np.asarray(inputs["ln_gamma"], dtype=np.float32)
    beta = np.asarray(inputs["ln_beta"], dtype=np.float32)

    eye = np.eye(D, dtype=np.float32)
    # Paths not needed for the actual parameterization of this module.
    assert np.array_equal(Wpk, eye), "general Wpk not supported"
    assert np.array_equal(Wpv @ 1.0, eye) and np.array_equal(Wpo, eye), \
        "general Wpv/Wpo not supported"
    assert np.all(gamma == 1.0) and np.all(beta == 0.0), \
        "general layernorm affine not supported"
    assert sim_bias == 0.0, "nonzero sim_bias not supported"

    mqk = (Wq.T @ Wk).astype(np.float32)
    use_mqk = not np.array_equal(mqk, eye)
    use_b1 = bool(np.any(b1 != 0.0))
    use_b2 = bool(np.any(b2 != 0.0))

    key = (use_mqk, use_b1, use_b2)
    if key not in _BUILD_CACHE:
        _BUILD_CACHE[key] = _build(*key)
    nc = _BUILD_CACHE[key]

    w1t = np.ascontiguousarray(W1.T)
    w2t = np.ascontiguousarray(W2.T)
    qbc = np.broadcast_to((lq * np.float32(HD ** -0.5)).astype(np.float32),
                          (128, D)).copy()
    triu = np.triu(np.ones((128, 128), dtype=np.float32))
    shf = np.zeros((128, 128), dtype=np.float32)
    shf[np.arange(1, 128), np.arange(0, 127)] = 1.0   # shf[k, m]=1 iff k==m+1
    e127 = np.zeros((128, 128), dtype=np.float32)
    e127[0, 127] = 1.0

    # actual length per batch: device f32->int cast rounds to nearest
    al = np.rint(lengths * np.float32(L)).astype(np.int64)
    lenmasks = [(np.arange(L) < al[b]).astype(np.float32) for b in range(B)]

    in_maps = []
    for c in range(N_CORES):
        b = c // SPLIT
        s_half = c % SPLIT
        m = {
            "hidden": hidden[b],
            "w1t": w1t,
            "w2t": w2t,
            "qbc": qbc,
            "lenmask": lenmasks[b],
            "iota_b": (s_half * SH + np.arange(SH, dtype=np.float32)
                       ).reshape(1, SH),
            "triu": triu,
            "shift1": shf,
            "e127": e127,
        }
        if use_mqk:
            m["mqk"] = mqk
        if use_b1:
            m["b1row"] = b1.reshape(1, D)
        if use_b2:
            m["b2row"] = b2.reshape(1, D)
        in_maps.append(m)

    res = run_bass_kernel_spmd(nc, in_maps, core_ids=list(range(N_CORES)))
    out = np.empty((B, L, D), dtype=np.float32)
    for b in range(B):
        for s_half in range(SPLIT):
            out[b, s_half * SH:(s_half + 1) * SH, :] = \
                res.results[b * SPLIT + s_half]["out"]
    return out


# revision 27
# speedup vs baseline: 1.1267x; 1.1267x over previous
"""Trainium2 Bass kernel for nn_BoundaryPredictor3 (segment_reduce).

Strategy (see test.py for the validation harness):
  - 8 cores: core c handles batch c//2 and output-segment half c%2.
  - Per core: boundary-MLP pipeline in f32 on-chip, exact replication of the
    executed-reference's float32 "dirt" semantics for boundaries
    (boundaries = (hard + p) - p, cumsum accumulated exactly then rounded
    once, f32-exact onehot compare), then segment softmax pooling via
    onehot matmuls on the PE.
  - Host only folds parameters (transposes, Wq.T@Wk, broadcast of the
    learned query) and computes the 4 length scalars.
"""
import sys
import os
sys.path.insert(0, '/opt/trn_rl_repo')

import numpy as np
from contextlib import ExitStack

B, L, D, H, HD = 4, 1024, 512, 8, 64
NT = L // 128            # 8 l-tiles
NC = D // 128            # 4 d-chunks
N_CORES = 8
SPLIT = 2                # s-halves per batch
SH = L // SPLIT          # 512 segment slots per core
ST = SH // 128           # 4 s-tiles per core
EPS_L2 = 1e-8
LN_EPS = 1e-5

_BUILD_CACHE = {}


def _build(use_mqk, use_b1, use_b2, dbg=False):
    import concourse.bass as bass
    import concourse.bacc as bacc
    import concourse.tile as tile
    from concourse import mybir
    from concourse.masks import make_identity

    F32 = mybir.dt.float32
    OP = mybir.AluOpType
    AF = mybir.ActivationFunctionType

    nc = bacc.Bacc("TRN2", target_bir_lowering=False, debug=False,
                   enable_asserts=True, num_devices=N_CORES)

    hid = nc.dram_tensor("hidden", [L, D], F32, kind="ExternalInput").ap()
    BF16D = mybir.dt.bfloat16
    w1h_d = nc.dram_tensor("w1h", [D, D], BF16D, kind="ExternalInput").ap()
    w1l_d = nc.dram_tensor("w1l", [D, D], BF16D, kind="ExternalInput").ap()
    w2h_d = nc.dram_tensor("w2h", [D, D], BF16D, kind="ExternalInput").ap()
    w2l_d = nc.dram_tensor("w2l", [D, D], BF16D, kind="ExternalInput").ap()
    qbc_d = nc.dram_tensor("qbc", [128, D], F32, kind="ExternalInput").ap()
    lm_d = nc.dram_tensor("lenmask", [L], F32, kind="ExternalInput").ap()
    iota_d = nc.dram_tensor("iota_b", [1, SH], F32, kind="ExternalInput").ap()
    triu_d = nc.dram_tensor("triu", [128, 128], F32, kind="ExternalInput").ap()
    shf_d = nc.dram_tensor("shift1", [128, 128], F32, kind="ExternalInput").ap()
    e127_d = nc.dram_tensor("e127", [128, 128], F32, kind="ExternalInput").ap()
    if use_mqk:
        mqk_d = nc.dram_tensor("mqk", [D, D], F32, kind="ExternalInput").ap()
    if use_b1:
        b1_d = nc.dram_tensor("b1row", [1, D], F32, kind="ExternalInput").ap()
    if use_b2:
        b2_d = nc.dram_tensor("b2row", [1, D], F32, kind="ExternalInput").ap()
    out_d = nc.dram_tensor("out", [SH, D], F32, kind="ExternalOutput").ap()
    if dbg:
        dbg_cos = nc.dram_tensor("dbg_cos", [128, NT], F32, kind="ExternalOutput").ap()
        dbg_seg = nc.dram_tensor("dbg_seg", [128, NT], F32, kind="ExternalOutput").ap()
        dbg_bex = nc.dram_tensor("dbg_bex", [128, NT], F32, kind="ExternalOutput").ap()
        dbg_E = nc.dram_tensor("dbg_E", [128, NT, H], F32, kind="ExternalOutput").ap()

    with ExitStack() as ctx:
        tc = ctx.enter_context(tile.TileContext(nc))
        cst = ctx.enter_context(tc.tile_pool(name="cst", bufs=1))
        big = ctx.enter_context(tc.tile_pool(name="big", bufs=1))
        wrk = ctx.enter_context(tc.tile_pool(name="wrk", bufs=2))
        sml = ctx.enter_context(tc.tile_pool(name="sml", bufs=2))
        psb = ctx.enter_context(tc.tile_pool(name="psb", bufs=4, space="PSUM"))
        pss = ctx.enter_context(tc.tile_pool(name="pss", bufs=2, space="PSUM"))

        # ---------------- constants ----------------
        ident = cst.tile([128, 128], F32, tag="ident")
        make_identity(nc, ident)
        BF16 = mybir.dt.bfloat16
        w1sh = cst.tile([128, NC, NC, 128], BF16, tag="w1sh")  # [k, kc, jt, m]
        w1sl = cst.tile([128, NC, NC, 128], BF16, tag="w1sl")
        w1rh = w1h_d.rearrange("(kc k) (jt m) -> k kc jt m", k=128, m=128)
        w1rl = w1l_d.rearrange("(kc k) (jt m) -> k kc jt m", k=128, m=128)
        nc.gpsimd.dma_start(out=w1sh, in_=w1rh)
        nc.gpsimd.dma_start(out=w1sl, in_=w1rl)
        w2sh = cst.tile([128, NC, D], BF16, tag="w2sh")        # [k, kc, n]
        w2sl = cst.tile([128, NC, D], BF16, tag="w2sl")
        nc.gpsimd.dma_start(out=w2sh, in_=w2h_d.rearrange("(kc k) n -> k kc n", k=128))
        nc.gpsimd.dma_start(out=w2sl, in_=w2l_d.rearrange("(kc k) n -> k kc n", k=128))
        if use_mqk:
            mqs = cst.tile([128, NC, NC, 128], F32, tag="mqs")
            nc.sync.dma_start(out=mqs, in_=mqk_d.rearrange(
                "(kc k) (jt m) -> k kc jt m", k=128, m=128))
        qbc = cst.tile([128, D], F32, tag="qbc")
        nc.gpsimd.dma_start(out=qbc, in_=qbc_d)
        lm8 = cst.tile([128, NT], F32, tag="lm8")            # col-major lenmask
        nc.sync.dma_start(out=lm8, in_=lm_d.rearrange("(t p) -> p t", p=128))
        triu = cst.tile([128, 128], F32, tag="triu")
        nc.gpsimd.dma_start(out=triu, in_=triu_d)
        shf = cst.tile([128, 128], F32, tag="shf")
        nc.gpsimd.dma_start(out=shf, in_=shf_d)
        e127 = cst.tile([128, 128], F32, tag="e127")
        nc.gpsimd.dma_start(out=e127, in_=e127_d)
        iota_r = cst.tile([1, SH], F32, tag="iota_r")
        nc.sync.dma_start(out=iota_r, in_=iota_d)
        ones_r128 = cst.tile([1, 128], F32, tag="ones_r128")
        nc.vector.memset(ones_r128, 1.0)
        ones_c = cst.tile([128, 1], F32, tag="ones_c")
        nc.vector.memset(ones_c, 1.0)
        if use_b1 or use_b2:
            ones_r512 = cst.tile([1, D], F32, tag="ones_r512")
            nc.vector.memset(ones_r512, 1.0)
        if use_b1:
            b1r = cst.tile([1, D], F32, tag="b1r")
            nc.sync.dma_start(out=b1r, in_=b1_d)
        if use_b2:
            b2r = cst.tile([1, D], F32, tag="b2r")
            nc.sync.dma_start(out=b2r, in_=b2_d)

        # iota broadcast [128, SH] via K=1 matmul
        ps_io = psb.tile([128, SH], F32, tag="ps_big")
        nc.tensor.matmul(ps_io, ones_r128, iota_r, start=True, stop=True)
        iota_b = cst.tile([128, SH], F32, tag="iota_b")
        nc.vector.tensor_copy(iota_b, ps_io)

        def newton_invsqrt(s_cols, seed, iters, name):
            """y ~= 1/sqrt(s_cols), Newton from constant seed (DVE only)."""
            y = sml.tile([128, NT], F32, tag=f"nw_y_{name}")
            nc.vector.memset(y, seed)
            for i in range(iters):
                t = sml.tile([128, NT], F32, tag=f"nw_t_{name}")
                nc.vector.tensor_tensor(out=t, in0=y, in1=y, op=OP.mult)
                nc.vector.tensor_tensor(out=t, in0=t, in1=s_cols, op=OP.mult)
                nc.vector.tensor_scalar(out=t, in0=t, scalar1=-0.5, scalar2=1.5,
                                        op0=OP.mult, op1=OP.add)
                y2 = sml.tile([128, NT], F32, tag=f"nw_y_{name}")
                nc.vector.tensor_tensor(out=y2, in0=y, in1=t, op=OP.mult)
                y = y2
            return y

        # ---------------- stage 1: load + norms ----------------
        X = big.tile([128, NT, D], F32, tag="X")
        hid_r = hid.rearrange("(t p) d -> p t d", p=128)
        for t in range(NT):
            eng = (nc.sync, nc.gpsimd)[t % 2]
            eng.dma_start(out=X[:, t, :], in_=hid_r[:, t, :])

        ss = sml.tile([128, NT], F32, tag="ss")
        for t in range(NT):
            scr = wrk.tile([128, D], F32, tag="scr_sqx")
            nc.scalar.activation(scr, X[:, t, :], AF.Square,
                                 accum_out=ss[:, t:t + 1])
        inv_c = newton_invsqrt(ss, 1.0 / np.sqrt(512.0), 4, "x")

        N8 = big.tile([128, NT, D], F32, tag="N8")
        for t in range(NT):
            nc.gpsimd.tensor_scalar(out=N8[:, t, :], in0=X[:, t, :],
                                    scalar1=inv_c[:, t:t + 1], scalar2=None,
                                    op0=OP.mult)

        # layernorm stats
        mv = sml.tile([128, NT, 2], F32, tag="mv")
        for t in range(NT):
            stt = wrk.tile([128, 6], F32, tag="bn_st")
            nc.vector.bn_stats(out=stt, in_=X[:, t, :])
            nc.vector.bn_aggr(out=mv[:, t, :], in_=stt)
        veps = sml.tile([128, NT], F32, tag="veps")
        nc.vector.tensor_scalar(out=veps, in0=mv[:, :, 1], scalar1=LN_EPS,
                                scalar2=None, op0=OP.add)
        rstd = newton_invsqrt(veps, 1.0, 4, "v")

        HN = big.tile([128, NT, D], F32, tag="HN")
        for t in range(NT):
            nc.vector.tensor_scalar(out=HN[:, t, :], in0=X[:, t, :],
                                    scalar1=mv[:, t, 0:1],
                                    scalar2=rstd[:, t:t + 1],
                                    op0=OP.subtract, op1=OP.mult)

        # ---------------- scores (DVE part, early: overlaps PE MLP) -----
        sc8 = sml.tile([128, NT, H], F32, tag="sc8")
        for t in range(NT):
            tmp = wrk.tile([128, D], F32, tag="scr_sc")
            nc.vector.tensor_tensor(out=tmp, in0=HN[:, t, :], in1=qbc, op=OP.mult)
            nc.vector.tensor_reduce(out=sc8[:, t, :],
                                    in_=tmp.rearrange("p (h j) -> p h j", h=H),
                                    axis=mybir.AxisListType.X, op=OP.add)

        # ------- stage 2: NT transposes -> bf16 hi/lo (for fast MLP) -------
        NTh = big.tile([128, NC, L], BF16, tag="NTh")
        NTl = big.tile([128, NC, L], BF16, tag="NTl")
        for dc in range(NC):
            for half in range(2):
                sl = slice(half * 512, (half + 1) * 512)
                pst = psb.tile([128, 512], F32, tag="ps_big")
                for i in range(4):
                    t = half * 4 + i
                    nc.tensor.transpose(pst[:, i * 128:(i + 1) * 128],
                                        N8[:, t, dc * 128:(dc + 1) * 128], ident)
                nc.scalar.copy(NTh[:, dc, sl], pst)
                h32 = wrk.tile([128, 512], F32, tag="scr_nth")
                nc.gpsimd.tensor_copy(h32, NTh[:, dc, sl])
                nc.vector.tensor_tensor(out=NTl[:, dc, sl], in0=pst, in1=h32,
                                        op=OP.subtract)

        # ------ stage 3: y1T = W1 @ n^T via 3-term bf16 hi/lo, gelu -------
        y1h = big.tile([128, NC, L], BF16, tag="y1h")
        y1l = big.tile([128, NC, L], BF16, tag="y1l")
        for jt in range(NC):
            for nch in range(2):
                sl = slice(nch * 512, (nch + 1) * 512)
                ps = psb.tile([128, 512], F32, tag="ps_big")
                nterms = 3 * NC + (1 if use_b1 else 0)
                i = 0
                for kc in range(NC):
                    for (wop, nop) in ((w1sh, NTh), (w1sh, NTl), (w1sl, NTh)):
                        nc.tensor.matmul(ps, wop[:, kc, jt, :], nop[:, kc, sl],
                                         start=(i == 0), stop=(i == nterms - 1))
                        i += 1
                if use_b1:
                    nc.tensor.matmul(ps, b1r[:, jt * 128:(jt + 1) * 128],
                                     ones_r512[:, 0:512], start=False, stop=True)
                g32 = wrk.tile([128, 512], F32, tag="scr_y1h")
                nc.scalar.activation(g32, ps, AF.Gelu)
                nc.vector.tensor_copy(y1h[:, jt, sl], g32)
                h32b = wrk.tile([128, 512], F32, tag="scr_y1b")
                nc.gpsimd.tensor_copy(h32b, y1h[:, jt, sl])
                nc.vector.tensor_tensor(out=y1l[:, jt, sl], in0=g32,
                                        in1=h32b, op=OP.subtract)

        # ------- E = exp(scores) and value/exp bf16 hi/lo splits (early) ----
        import concourse.bass as bass_mod
        BF16 = mybir.dt.bfloat16
        E8 = sml.tile([128, NT, H], F32, tag="E8")
        nc.scalar.activation(E8, sc8, AF.Exp)
        E8h = sml.tile([128, NT, H], BF16, tag="E8h")
        E8l = sml.tile([128, NT, H], BF16, tag="E8l")
        nc.vector.tensor_copy(E8h, E8)
        eh32 = sml.tile([128, NT, H], F32, tag="eh32")
        nc.vector.tensor_copy(eh32, E8h)
        el32 = sml.tile([128, NT, H], F32, tag="el32")
        nc.vector.tensor_tensor(out=el32, in0=E8, in1=eh32, op=OP.subtract)
        nc.vector.tensor_copy(E8l, el32)
        Xp = big.tile([128, NT, D], F32, tag="X")
        Xph = big.tile([128, NT, D], BF16, tag="Xph")
        Xpl = big.tile([128, NT, D], BF16, tag="Xpl")
        for t in range(NT):
            e_t = E8[:, t, :]
            ebc = bass_mod.AP(tensor=e_t.tensor, offset=e_t.offset,
                              ap=[e_t.ap[0], [e_t.ap[1][0], H], [0, HD]])
            nc.vector.tensor_tensor(
                out=Xp[:, t, :].rearrange("p (h j) -> p h j", h=H),
                in0=HN[:, t, :].rearrange("p (h j) -> p h j", h=H),
                in1=ebc, op=OP.mult)
            nc.vector.tensor_copy(Xph[:, t, :], Xp[:, t, :])
            h32 = wrk.tile([128, D], F32, tag="scr_hi")
            nc.gpsimd.tensor_copy(h32, Xph[:, t, :])
            lo32 = wrk.tile([128, D], F32, tag="scr_lo")
            nc.vector.tensor_tensor(out=lo32, in0=Xp[:, t, :], in1=h32,
                                    op=OP.subtract)
            nc.vector.tensor_copy(Xpl[:, t, :], lo32)

        # ---------------- stage 4: y2 + residual ----------------
        R8 = big.tile([128, NT, D], F32, tag="R8")
        for lt in range(NT):
            lsl = slice(lt * 128, (lt + 1) * 128)
            ps = psb.tile([128, 512], F32, tag="ps_big")
            nterms = 3 * NC + (1 if use_b2 else 0)
            i = 0
            for kc in range(NC):
                for (aop, wop) in ((y1h, w2sh), (y1h, w2sl), (y1l, w2sh)):
                    nc.tensor.matmul(ps, aop[:, kc, lsl], wop[:, kc, :],
                                     start=(i == 0), stop=(i == nterms - 1))
                    i += 1
            if use_b2:
                nc.tensor.matmul(ps, ones_r128, b2r, start=False, stop=True)
            nc.vector.tensor_tensor(out=R8[:, lt, :], in0=ps, in1=N8[:, lt, :],
                                    op=OP.add)

        # ---------------- stage 5: l2norm scalars of R ----------------
        ssr = sml.tile([128, NT], F32, tag="ssr")
        for t in range(NT):
            scr = wrk.tile([128, D], F32, tag="scr_sqr")
            nc.scalar.activation(scr, R8[:, t, :], AF.Square,
                                 accum_out=ssr[:, t:t + 1])
        invr = newton_invsqrt(ssr, 1.0, 4, "r")

        # ---------------- stage 6: RT transposes ----------------
        RT = big.tile([128, NC, L], F32, tag="RT")
        for dc in range(NC):
            for half in range(2):
                pst = psb.tile([128, 512], F32, tag="ps_big")
                for i in range(4):
                    t = half * 4 + i
                    nc.tensor.transpose(pst[:, i * 128:(i + 1) * 128],
                                        R8[:, t, dc * 128:(dc + 1) * 128], ident)
                nc.scalar.copy(RT[:, dc, half * 512:(half + 1) * 512], pst)

        # ---------------- stage 7: optional Mqk fold ----------------
        if use_mqk:
            ART = big.tile([128, NC, L], F32, tag="ART")
            for jt in range(NC):
                for nch in range(2):
                    ps = psb.tile([128, 512], F32, tag="ps_big")
                    for kc in range(NC):
                        nc.tensor.matmul(ps, mqs[:, kc, jt, :],
                                         RT[:, kc, nch * 512:(nch + 1) * 512],
                                         start=(kc == 0), stop=(kc == NC - 1))
                    nc.scalar.copy(ART[:, jt, nch * 512:(nch + 1) * 512], ps)
        else:
            ART = RT

        # ---------------- stage 8: cos ----------------
        prod = big.tile([128, NC, L], F32, tag="R8")
        for dc in range(NC):
            for th in range(2):
                lo, hi_ = th * 512, min((th + 1) * 512, L - 1)
                nc.vector.tensor_tensor(out=prod[:, dc, lo:hi_],
                                        in0=ART[:, dc, lo:hi_],
                                        in1=RT[:, dc, lo + 1:hi_ + 1], op=OP.mult)
            nc.vector.memset(prod[:, dc, L - 1:L], 0.0)
        ps_raw = pss.tile([128, NT], F32, tag="ps_sml")
        for t in range(NT):
            for dc in range(NC):
                nc.tensor.matmul(ps_raw[:, t:t + 1],
                                 prod[:, dc, t * 128:(t + 1) * 128], ones_c,
                                 start=(dc == 0), stop=(dc == NC - 1))
        ps_shift = pss.tile([128, NT], F32, tag="ps_sml")
        nc.tensor.matmul(ps_shift, shf, invr, start=True, stop=False)
        nc.tensor.matmul(ps_shift[:, 0:NT - 1], e127, invr[:, 1:NT],
                         start=False, stop=True)
        cosr = sml.tile([128, NT], F32, tag="cosr")
        nc.vector.tensor_tensor(out=cosr, in0=ps_raw, in1=invr, op=OP.mult)
        cos_c = sml.tile([128, NT], F32, tag="cos_c")
        nc.vector.tensor_tensor(out=cos_c, in0=cosr, in1=ps_shift, op=OP.mult)

        # ---------------- stage 9: boundary chain (exact f32 dirt) --------
        probs = sml.tile([128, NT], F32, tag="probs")
        # (cos - 1) * -0.5 == fl((1-cos)*0.5) bitwise
        nc.vector.tensor_scalar(out=probs, in0=cos_c, scalar1=1.0, scalar2=-0.5,
                                op0=OP.subtract, op1=OP.mult)
        nc.vector.tensor_scalar(out=probs, in0=probs, scalar1=0.0, scalar2=1.0,
                                op0=OP.max, op1=OP.min)
        # pad l = L-1 is handled upstream: prod[:, :, L-1] is zeroed, so
        # cos[L-1] = 0 -> probs = 0.5 exactly -> hard = 0, b_exec = 0.
        hard = sml.tile([128, NT], F32, tag="hard")
        nc.vector.tensor_scalar(out=hard, in0=probs, scalar1=0.5, scalar2=None,
                                op0=OP.is_gt)
        thp = sml.tile([128, NT], F32, tag="thp")
        nc.vector.tensor_tensor(out=thp, in0=hard, in1=probs, op=OP.add)
        bexec = sml.tile([128, NT], F32, tag="bexec")
        nc.vector.tensor_tensor(out=bexec, in0=thp, in1=probs, op=OP.subtract)
        dirt = sml.tile([128, NT], F32, tag="dirt")
        nc.vector.tensor_tensor(out=dirt, in0=hard, in1=bexec, op=OP.subtract)

        def carry_excl(cols, name):
            """[1, NT] exclusive cumsum of column totals of cols [128, NT]."""
            ps_tot = pss.tile([1, NT], F32, tag="ps_tot")
            nc.tensor.matmul(ps_tot, ones_c, cols, start=True, stop=True)
            a = sml.tile([1, NT], F32, tag=f"ce_a_{name}")
            nc.vector.memset(a[:, 0:1], 0.0)
            nc.vector.tensor_copy(a[:, 1:NT], ps_tot[:, 0:NT - 1])
            for sh_ in (1, 2, 4):
                b = sml.tile([1, NT], F32, tag=f"ce_a_{name}")
                nc.vector.tensor_copy(b[:, 0:sh_], a[:, 0:sh_])
                nc.vector.tensor_tensor(out=b[:, sh_:NT], in0=a[:, sh_:NT],
                                        in1=a[:, 0:NT - sh_], op=OP.add)
                a = b
            return a

        carH = carry_excl(hard, "h")
        carD = carry_excl(dirt, "d")
        psK = pss.tile([128, NT], F32, tag="ps_sml")
        nc.tensor.matmul(psK, triu, hard, start=True, stop=False)
        nc.tensor.matmul(psK, ones_r128, carH, start=False, stop=True)
        psD = pss.tile([128, NT], F32, tag="ps_sml")
        nc.tensor.matmul(psD, triu, dirt, start=True, stop=False)
        nc.tensor.matmul(psD, ones_r128, carD, start=False, stop=True)
        sbK = sml.tile([128, NT], F32, tag="sbK")
        nc.vector.tensor_copy(sbK, psK)
        cum = sml.tile([128, NT], F32, tag="cum")
        nc.vector.tensor_tensor(out=cum, in0=sbK, in1=psD, op=OP.subtract)
        seg = sml.tile([128, NT], F32, tag="seg")
        nc.vector.tensor_tensor(out=seg, in0=cum, in1=bexec, op=OP.subtract)
        # mask positions beyond actual length: push seg out of range
        lmB = sml.tile([128, NT], F32, tag="lmB")
        nc.vector.tensor_scalar(out=lmB, in0=lm8, scalar1=-1e9, scalar2=1e9,
                                op0=OP.mult, op1=OP.add)
        segm = sml.tile([128, NT], F32, tag="segm")
        nc.vector.tensor_tensor(out=segm, in0=seg, in1=lmB, op=OP.add)
        if dbg:
            nc.sync.dma_start(out=dbg_cos, in_=cos_c)
            nc.sync.dma_start(out=dbg_seg, in_=seg)
            nc.sync.dma_start(out=dbg_bex, in_=bexec)

        if dbg:
            nc.sync.dma_start(out=dbg_E, in_=E8)

        # ---------------- stage 12: onehot + pooling ----------------
        oh = big.tile([128, NT, SH], BF16, tag="N8")
        for t in range(NT):
            eng = (nc.vector, nc.gpsimd)[t % 2]
            eng.tensor_scalar(out=oh[:, t, :], in0=iota_b,
                              scalar1=segm[:, t:t + 1], scalar2=None,
                              op0=OP.is_equal)
        for st in range(ST):
            psP = psb.tile([128, 512], F32, tag="ps_big")
            psZ = pss.tile([128, H], F32, tag="ps_sml")
            for t in range(NT):
                nc.tensor.matmul(psP, oh[:, t, st * 128:(st + 1) * 128],
                                 Xph[:, t, :], start=(t == 0), stop=False)
            for t in range(NT):
                nc.tensor.matmul(psP, oh[:, t, st * 128:(st + 1) * 128],
                                 Xpl[:, t, :], start=False, stop=(t == NT - 1))
            for t in range(NT):
                nc.tensor.matmul(psZ, oh[:, t, st * 128:(st + 1) * 128],
                                 E8h[:, t, :], start=(t == 0), stop=False)
            for t in range(NT):
                nc.tensor.matmul(psZ, oh[:, t, st * 128:(st + 1) * 128],
                                 E8l[:, t, :], start=False, stop=(t == NT - 1))
            ispos = sml.tile([128, H], F32, tag="ispos")
            nc.vector.tensor_scalar(out=ispos, in0=psZ, scalar1=0.0,
                                    scalar2=None, op0=OP.is_gt)
            notp = sml.tile([128, H], F32, tag="notp")
            nc.vector.tensor_scalar(out=notp, in0=ispos, scalar1=-1.0,
                                    scalar2=1.0, op0=OP.mult, op1=OP.add)
            zsafe = sml.tile([128, H], F32, tag="zsafe")
            nc.vector.tensor_tensor(out=zsafe, in0=psZ, in1=notp, op=OP.add)
            rz = sml.tile([128, H], F32, tag="rz")
            nc.vector.reciprocal(rz, zsafe)
            nc.vector.tensor_tensor(out=rz, in0=rz, in1=ispos, op=OP.mult)
            pooled = wrk.tile([128, D], F32, tag="scr_po")
            rzb = bass_mod.AP(tensor=rz.tensor, offset=rz.offset,
                              ap=[rz.ap[0], [rz.ap[1][0], H], [0, HD]])
            nc.vector.tensor_tensor(
                out=pooled.rearrange("p (h j) -> p h j", h=H),
                in0=psP.rearrange("p (h j) -> p h j", h=H),
                in1=rzb, op=OP.mult)
            nc.sync.dma_start(out=out_d[st * 128:(st + 1) * 128, :], in_=pooled)

    nc.finalize()
    return nc


def _prepare(inputs, dbg=False):
    hidden = np.ascontiguousarray(np.asarray(inputs["hidden"], dtype=np.float32))
    lengths = np.asarray(inputs["lengths"], dtype=np.float32)
    W1 = np.asarray(inputs["W1"], dtype=np.float32)
    b1 = np.asarray(inputs["b1"], dtype=np.float32)
    W2 = np.asarray(inputs["W2"], dtype=np.float32)
    b2 = np.asarray(inputs["b2"], dtype=np.float32)
    Wq = np.asarray(inputs["Wq"], dtype=np.float32)
    Wk = np.asarray(inputs["Wk"], dtype=np.float32)
    sim_bias = float(np.asarray(inputs["sim_bias"], dtype=np.float32))
    lq = np.asarray(inputs["learned_query"], dtype=np.float32)
    Wpk = np.asarray(inputs["Wpk"], dtype=np.float32)
    Wpv = np.asarray(inputs["Wpv"], dtype=np.float32)
    Wpo = np.asarray(inputs["Wpo"], dtype=np.float32)
    gamma = np.asarray(inputs["ln_gamma"], dtype=np.float32)
    beta = np.asarray(inputs["ln_beta"], dtype=np.float32)

    eye = np.eye(D, dtype=np.float32)
    # Paths not needed for the actual parameterization of this module.
    assert np.array_equal(Wpk, eye), "general Wpk not supported"
    assert np.array_equal(Wpv, eye) and np.array_equal(Wpo, eye), \
        "general Wpv/Wpo not supported"
    assert np.all(gamma == 1.0) and np.all(beta == 0.0), \
        "general layernorm affine not supported"
    assert sim_bias == 0.0, "nonzero sim_bias not supported"

    mqk = (Wq.T @ Wk).astype(np.float32)
    use_mqk = not np.array_equal(mqk, eye)
    use_b1 = bool(np.any(b1 != 0.0))
    use_b2 = bool(np.any(b2 != 0.0))

    key = (use_mqk, use_b1, use_b2, dbg)
    if key not in _BUILD_CACHE:
        _BUILD_CACHE[key] = _build(*key[:3], dbg=dbg)
    nc = _BUILD_CACHE[key]

    import ml_dtypes
    def split16(a):
        hi = a.astype(ml_dtypes.bfloat16)
        lo = (a - hi.astype(np.float32)).astype(ml_dtypes.bfloat16)
        return np.ascontiguousarray(hi), np.ascontiguousarray(lo)
    w1h, w1l = split16(np.ascontiguousarray(W1.T))
    w2h, w2l = split16(np.ascontiguousarray(W2.T))
    qbc = np.broadcast_to((lq * np.float32(HD ** -0.5)).astype(np.float32),
                          (128, D)).copy()
    triu = np.triu(np.ones((128, 128), dtype=np.float32))
    shf = np.zeros((128, 128), dtype=np.float32)
    shf[np.arange(1, 128), np.arange(0, 127)] = 1.0   # shf[k, m]=1 iff k==m+1
    e127 = np.zeros((128, 128), dtype=np.float32)
    e127[0, 127] = 1.0

    # actual length per batch: device f32->int cast rounds to nearest
    al = np.rint(lengths * np.float32(L)).astype(np.int64)
    lenmasks = [(np.arange(L) < al[b]).astype(np.float32) for b in range(B)]

    in_maps = []
    for c in range(N_CORES):
        b = c // SPLIT
        s_half = c % SPLIT
        m = {
            "hidden": hidden[b],
            "w1h": w1h, "w1l": w1l,
            "w2h": w2h, "w2l": w2l,
            "qbc": qbc,
            "lenmask": lenmasks[b],
            "iota_b": (s_half * SH + np.arange(SH, dtype=np.float32)
                       ).reshape(1, SH),
            "triu": triu,
            "shift1": shf,
            "e127": e127,
        }
        if use_mqk:
            m["mqk"] = mqk
        if use_b1:
            m["b1row"] = b1.reshape(1, D)
        if use_b2:
            m["b2row"] = b2.reshape(1, D)
        in_maps.append(m)
    return nc, in_maps


def kernel(**inputs):
    from concourse.bass_utils import run_bass_kernel_spmd

    nc, in_maps = _prepare(inputs)
    res = run_bass_kernel_spmd(nc, in_maps, core_ids=list(range(N_CORES)))
    out = np.empty((B, L, D), dtype=np.float32)
    for b in range(B):
        for s_half in range(SPLIT):
            out[b, s_half * SH:(s_half + 1) * SH, :] = \
                res.results[b * SPLIT + s_half]["out"]
    return out


def run_traced(**inputs):
    """Run with NTFF tracing; returns exec_time_ns (or None)."""
    from concourse.bass_utils import run_bass_kernel_spmd

    nc, in_maps = _prepare(inputs)
    res = run_bass_kernel_spmd(nc, in_maps, core_ids=list(range(N_CORES)),
                               trace=True)
    return res.exec_time_ns


def run_debug(**inputs):
    """Run debug build; returns per-core dicts incl cos/seg/bexec/E."""
    from concourse.bass_utils import run_bass_kernel_spmd

    nc, in_maps = _prepare(inputs, dbg=True)
    res = run_bass_kernel_spmd(nc, in_maps, core_ids=list(range(N_CORES)))
    return res.results
